# revision 3
# baseline (speedup 1.0000x reference)
"""Trainium2 Bass kernel for masked depthwise conv1d with fake quantization.

Problem: x[32,1024,2048] f32, lens[32] i32, weight[128,1,33] f32.
  - zero x beyond per-batch lens
  - global activation scale s_a = max|masked x| / 127, per-channel weight
    scale s_w = max|w| / 127, symmetric 8-bit fake quant of both
  - depthwise conv (128 heads, K=33, pad 16), out = (out, out_lens, out_scale)

Sharding: data-parallel over batch, 4 batches/core on 8 cores. The global
activation max needs one 4-byte AllReduce(max).

Device algorithm (per core), all in the DMA-friendly [channels=128, L] layout:
  pass 1: stream x, mask via precomputed iota<len compare, abs-max reduce
  AllReduce(max) -> s_a
  pass 2: stream x, q = round(x * (1/s_a)) via the +-1.5*2^23 magic-number
    round (bit-exact round-half-even), mask, cast to bf16 (quantized values
    are integers <= ~129, exact in bf16). Depthwise conv runs on the PE as 33
    PSUM-accumulating matmuls with stationary diag(qw[:,k]) and the moving
    operand a k-shifted slice of the padded bf16 tile; integer products
    accumulate exactly in fp32 PSUM. ScalarE copies PSUM->SBUF applying the
    per-channel scale s_a*s_w[g].
"""
import sys

sys.path.insert(0, "/opt/trn_rl_repo")

import numpy as np

import concourse.bass as bass
import concourse.tile as tile
from concourse import bacc, bass_isa, mybir
from concourse.bass_utils import run_bass_kernel_spmd

N_CORES = 8
B = 32
C = 1024
L = 2048
HEADS = 128
K = 33
PAD = 16
B_LOC = B // N_CORES          # batches per core
ROWS = B_LOC * (C // HEADS)   # [128, L] row-tiles per core
CH = 512                      # PSUM chunk (one bank of f32)
NCH = L // CH
PADL = L + 2 * PAD
CMAGIC = float(1.5 * 2 ** 23)
F32 = mybir.dt.float32
BF16 = mybir.dt.bfloat16
I32 = mybir.dt.int32

_COMPILED = None
LAST_RESULTS = None


def _build():
    nc = bacc.Bacc("TRN2", target_bir_lowering=False, debug=False,
                   num_devices=N_CORES)
    x_in = nc.dram_tensor("x", [B_LOC, C, L], F32, kind="ExternalInput")
    lens_in = nc.dram_tensor("lens", [1, B_LOC], I32, kind="ExternalInput")
    w_in = nc.dram_tensor("w", [HEADS, 1, K], F32, kind="ExternalInput")
    out = nc.dram_tensor("out", [B_LOC, C, L], F32, kind="ExternalOutput")
    oscale = nc.dram_tensor("oscale", [HEADS, 1], F32, kind="ExternalOutput")

    with tile.TileContext(nc) as tc:
        with tc.tile_pool(name="const", bufs=1) as cpool, \
             tc.tile_pool(name="xload", bufs=3) as xpool, \
             tc.tile_pool(name="work", bufs=2) as wpool, \
             tc.tile_pool(name="qpad", bufs=3) as qpool, \
             tc.tile_pool(name="outsb", bufs=6) as opool, \
             tc.tile_pool(name="psum", bufs=2, space="PSUM") as ppool, \
             tc.tile_pool(name="dram", bufs=1, space="DRAM") as dpool:

            # ---- weight prep: s_w, quantized diag matrices ----
            wt = cpool.tile([HEADS, K], F32)
            nc.sync.dma_start(wt[:], w_in[:, 0, :])
            wmax = cpool.tile([HEADS, 1], F32)
            nc.vector.tensor_reduce(wmax[:], wt[:], mybir.AxisListType.X,
                                    mybir.AluOpType.max,
                                    apply_absolute_value=True)
            s_w = cpool.tile([HEADS, 1], F32)
            nc.vector.tensor_scalar(s_w[:], wmax[:], 1.0 / 127.0, None,
                                    mybir.AluOpType.mult)
            inv_w = cpool.tile([HEADS, 1], F32)
            nc.vector.reciprocal(inv_w[:], s_w[:])
            qw = cpool.tile([HEADS, K], F32)
            nc.vector.tensor_scalar(qw[:], wt[:], inv_w[:], CMAGIC,
                                    mybir.AluOpType.mult, mybir.AluOpType.add)
            nc.vector.tensor_scalar(qw[:], qw[:], CMAGIC, None,
                                    mybir.AluOpType.subtract)
            io2 = cpool.tile([128, 128], I32)
            nc.gpsimd.iota(io2[:], [[1, 128]], channel_multiplier=-1)
            ident = cpool.tile([128, 128], BF16)
            nc.vector.tensor_scalar(ident[:], io2[:], 0.0, None,
                                    mybir.AluOpType.is_equal)
            diag = cpool.tile([128, K * 128], BF16)
            for k in range(K):
                nc.vector.tensor_scalar(diag[:, k * 128:(k + 1) * 128],
                                        ident[:], qw[:, k:k + 1], None,
                                        mybir.AluOpType.mult)

            # ---- masks from lens ----
            iol = cpool.tile([128, L], I32)
            nc.gpsimd.iota(iol[:], [[1, L]], channel_multiplier=0)
            iof = cpool.tile([128, L], F32)
            nc.vector.tensor_copy(iof[:], iol[:])
            lnp = cpool.tile([1, B_LOC], I32)
            nc.sync.dma_start(lnp[:], lens_in[:])
            lnb = cpool.tile([128, B_LOC], I32)
            nc.gpsimd.partition_broadcast(lnb[:], lnp[:])
            lnf = cpool.tile([128, B_LOC], F32)
            nc.vector.tensor_copy(lnf[:], lnb[:])
            mask_f = cpool.tile([128, B_LOC * L], F32)
            mask_h = cpool.tile([128, B_LOC * L], BF16)
            for b in range(B_LOC):
                nc.vector.tensor_scalar(mask_f[:, b * L:(b + 1) * L], iof[:],
                                        lnf[:, b:b + 1], None,
                                        mybir.AluOpType.is_lt)
                nc.vector.tensor_scalar(mask_h[:, b * L:(b + 1) * L], iof[:],
                                        lnf[:, b:b + 1], None,
                                        mybir.AluOpType.is_lt)

            # ---- pass 1: masked global abs-max ----
            stats = cpool.tile([128, ROWS], F32)
            for r in range(ROWS):
                b, j = divmod(r, C // HEADS)
                xt = xpool.tile([128, L], F32, tag="x1")
                nc.sync.dma_start(xt[:], x_in[b, j * 128:(j + 1) * 128, :])
                xm = wpool.tile([128, L], F32, tag="xm")
                nc.vector.tensor_tensor(xm[:], xt[:],
                                        mask_f[:, b * L:(b + 1) * L],
                                        mybir.AluOpType.mult)
                nc.vector.tensor_reduce(stats[:, r:r + 1], xm[:],
                                        mybir.AxisListType.X,
                                        mybir.AluOpType.max,
                                        apply_absolute_value=True)
            lmax = cpool.tile([128, 1], F32)
            nc.vector.tensor_reduce(lmax[:], stats[:], mybir.AxisListType.X,
                                    mybir.AluOpType.max)
            pmax = cpool.tile([128, 1], F32)
            nc.gpsimd.partition_all_reduce(pmax[:], lmax[:], 128,
                                           bass_isa.ReduceOp.max)
            cc_in = dpool.tile([1, 1], F32)
            cc_out = dpool.tile([1, 1], F32)
            nc.sync.dma_start(cc_in[:], pmax[0:1, :])
            nc.gpsimd.collective_compute(
                "AllReduce", mybir.AluOpType.max,
                replica_groups=[list(range(N_CORES))],
                ins=[cc_in.opt()], outs=[cc_out.opt()],
            )
            gsb = cpool.tile([1, 1], F32)
            nc.sync.dma_start(gsb[:], cc_out[:])
            gbc = cpool.tile([128, 1], F32)
            nc.gpsimd.partition_broadcast(gbc[:], gsb[:])
            s_a = cpool.tile([128, 1], F32)
            nc.vector.tensor_scalar(s_a[:], gbc[:], 1.0 / 127.0, None,
                                    mybir.AluOpType.mult)
            inv_a = cpool.tile([128, 1], F32)
            nc.vector.reciprocal(inv_a[:], s_a[:])
            s_fin = cpool.tile([128, 1], F32)
            nc.vector.tensor_tensor(s_fin[:], s_a[:], s_w[:],
                                    mybir.AluOpType.mult)
            nc.sync.dma_start(oscale[:], s_fin[:])

            # ---- pass 2: quantize + depthwise conv ----
            for r in range(ROWS):
                b, j = divmod(r, C // HEADS)
                xt2 = xpool.tile([128, L], F32, tag="x2")
                nc.sync.dma_start(xt2[:], x_in[b, j * 128:(j + 1) * 128, :])
                t1 = wpool.tile([128, L], F32, tag="t1")
                nc.vector.tensor_scalar(t1[:], xt2[:], inv_a[:], CMAGIC,
                                        mybir.AluOpType.mult,
                                        mybir.AluOpType.add)
                qb = wpool.tile([128, L], BF16, tag="qb")
                nc.vector.tensor_scalar(qb[:], t1[:], CMAGIC, None,
                                        mybir.AluOpType.subtract)
                xp = qpool.tile([128, PADL], BF16, tag="xp")
                nc.gpsimd.memset(xp[:, 0:PAD], 0)
                nc.gpsimd.memset(xp[:, PAD + L:PADL], 0)
                nc.vector.tensor_tensor(xp[:, PAD:PAD + L], qb[:],
                                        mask_h[:, b * L:(b + 1) * L],
                                        mybir.AluOpType.mult)
                pcs = [ppool.tile([128, CH], F32, tag=f"ps{c}",
                                  name=f"ps{c}_{r}")
                       for c in range(NCH)]
                for k in range(K):
                    for c in range(NCH):
                        nc.tensor.matmul(
                            pcs[c][:],
                            diag[:, k * 128:(k + 1) * 128],
                            xp[:, c * CH + k: c * CH + k + CH],
                            start=(k == 0), stop=(k == K - 1),
                        )
                for c in range(NCH):
                    ob = opool.tile([128, CH], F32, tag="ob")
                    nc.scalar.activation(ob[:], pcs[c][:],
                                         mybir.ActivationFunctionType.Copy,
                                         scale=s_fin[:])
                    nc.sync.dma_start(
                        out[b, j * 128:(j + 1) * 128, c * CH:(c + 1) * CH],
                        ob[:])

    nc.compile()
    return nc


def kernel(x: np.ndarray, lens: np.ndarray, weight: np.ndarray):
    global _COMPILED, LAST_RESULTS
    if _COMPILED is None:
        _COMPILED = _build()
    nc = _COMPILED

    x = np.ascontiguousarray(x, dtype=np.float32)
    lens = np.ascontiguousarray(lens, dtype=np.int32)
    weight = np.ascontiguousarray(weight, dtype=np.float32)

    in_maps = []
    for i in range(N_CORES):
        in_maps.append({
            "x": x[i * B_LOC:(i + 1) * B_LOC],
            "lens": lens[i * B_LOC:(i + 1) * B_LOC].reshape(1, B_LOC),
            "w": weight,
        })
    import os
    trace = bool(os.environ.get("KERNEL_TRACE"))
    LAST_RESULTS = run_bass_kernel_spmd(nc, in_maps, list(range(N_CORES)),
                                        trace=trace)
    res = LAST_RESULTS.results
    out = np.concatenate([res[i]["out"] for i in range(N_CORES)], axis=0)
    out_scale = res[0]["oscale"].reshape(-1)
    out_lens = ((lens + 2 * PAD - (K - 1) - 1) // 1 + 1).astype(np.int32)
    return out, out_lens, out_scale


# revision 7
# speedup vs baseline: 11468.5945x; 11468.5945x over previous
"""v3: channel-sharded Toeplitz band-matmul depthwise conv, overlapped phases.

Layout per core: 16 heads x 256 rows (= 32 batches x 8 row-blocks) x 2048 L.
Quantized bf16 x goes to TWO half-row DRAM scratches (so the banded-conv pass
for half 0 overlaps the quant pass for half 1), is transpose-loaded via the
xbar DMA as [L-on-partitions, rows-free], convolved on the PE as banded
matmuls (stationary = 128x128 Toeplitz band of quantized weights, host-built),
PE-transposed back to natural layout with the per-head scale applied on the
way out. Global activation max via one 4-byte AllReduce.
"""
import sys

sys.path.insert(0, "/opt/trn_rl_repo")

import numpy as np
import ml_dtypes

import concourse.bass as bass
import concourse.tile as tile
from concourse import bacc, bass_isa, mybir
from concourse.bass_utils import run_bass_kernel_spmd

N_CORES = 8
B = 32
C = 1024
L = 2048
HEADS = 128
K = 33
PAD = 16
GLOC = HEADS // N_CORES      # 16 heads per core
J = C // HEADS               # 8 row-blocks per batch
HB = B // 2                  # 16 batches per half
HROWS = HB * J               # 128 rows per half (b-major within half)
PADL = 2176                  # 17 chunks of 128 (>= L + 2*PAD = 2080)
NCHK = PADL // 128           # 17 input chunks
OCHK = L // 128              # 16 output chunks
CMAGIC = float(1.5 * 2 ** 23)
F32 = mybir.dt.float32
BF16 = mybir.dt.bfloat16
I32 = mybir.dt.int32

_COMPILED = None
LAST_RESULTS = None


def _build(single=False, p1_tt_pool=True, p2a_tt_pool=False,
           do_p1=True, do_p2a=True, do_p2b=True):
    nc = bacc.Bacc("TRN2", target_bir_lowering=False, debug=False,
                   num_devices=1 if single else N_CORES)
    x_in = nc.dram_tensor("x", [B, J, GLOC, L], F32, kind="ExternalInput")
    lens_in = nc.dram_tensor("lens", [1, B], I32, kind="ExternalInput")
    bands_in = nc.dram_tensor("bands", [128, GLOC, 2, 128], BF16,
                              kind="ExternalInput")
    sw_in = nc.dram_tensor("sw", [1, GLOC], F32, kind="ExternalInput")
    out = nc.dram_tensor("out", [B, J, GLOC, L], F32, kind="ExternalOutput")
    oscale = nc.dram_tensor("oscale", [1, GLOC], F32, kind="ExternalOutput")

    tt1 = nc.gpsimd if p1_tt_pool else nc.vector
    tt2 = nc.gpsimd if p2a_tt_pool else nc.vector

    with tile.TileContext(nc) as tc:
        with tc.tile_pool(name="const", bufs=1) as cpool, \
             tc.tile_pool(name="xload", bufs=3) as xpool, \
             tc.tile_pool(name="work", bufs=3) as wpool, \
             tc.tile_pool(name="qpad", bufs=3) as qpool, \
             tc.tile_pool(name="xt", bufs=3) as tpool, \
             tc.tile_pool(name="obt", bufs=6) as otpool, \
             tc.tile_pool(name="obuf", bufs=3) as opool, \
             tc.tile_pool(name="psum", bufs=2, space="PSUM") as ppool, \
             tc.tile_pool(name="psumt", bufs=4, space="PSUM") as ptpool, \
             tc.tile_pool(name="dram", bufs=1, space="DRAM") as dpool:

            bands = cpool.tile([128, GLOC, 2, 128], BF16)
            nc.sync.dma_start(bands[:], bands_in[:])
            identf = cpool.tile([128, 128], F32)
            ioi = cpool.tile([128, 128], I32)
            nc.gpsimd.iota(ioi[:], [[1, 128]], channel_multiplier=-1)
            nc.vector.tensor_scalar(identf[:], ioi[:], 0.0, None,
                                    mybir.AluOpType.is_equal)
            iol = cpool.tile([128, L], I32)
            nc.gpsimd.iota(iol[:], [[1, L]], channel_multiplier=0)
            iof = cpool.tile([128, L], F32)
            nc.vector.tensor_copy(iof[:], iol[:])
            lnp = cpool.tile([1, B], I32)
            nc.sync.dma_start(lnp[:], lens_in[:])
            lnb = cpool.tile([128, B], I32)
            nc.gpsimd.partition_broadcast(lnb[:], lnp[:])
            lnf = cpool.tile([128, B], F32)
            nc.vector.tensor_copy(lnf[:], lnb[:])
            sw_sb = cpool.tile([1, GLOC], F32)
            nc.sync.dma_start(sw_sb[:], sw_in[:])

            xq_h = [dpool.tile([GLOC, HROWS, PADL], BF16, name=f"xqh{h}")
                    for h in range(2)]

            # ---- pass 1: masked global abs-max ----
            stats = cpool.tile([128, B], F32)
            nc.vector.memset(stats[:], 1.0)
            for b in range(B if do_p1 else 0):
                xt1 = xpool.tile([128, L], F32, tag="x1", name=f"x1_{b}")
                nc.sync.dma_start(xt1[:], x_in[b])
                mkf = wpool.tile([128, L], F32, tag="mkf", name=f"mkf_{b}")
                nc.vector.tensor_scalar(mkf[:], iof[:], lnf[:, b:b + 1], None,
                                        mybir.AluOpType.is_lt)
                xm = wpool.tile([128, L], F32, tag="xm", name=f"xm_{b}")
                eng = tt1 if b % 2 == 0 else nc.vector
                eng.tensor_tensor(xm[:], xt1[:], mkf[:], mybir.AluOpType.mult)
                nc.vector.tensor_reduce(stats[:, b:b + 1], xm[:],
                                        mybir.AxisListType.X,
                                        mybir.AluOpType.max,
                                        apply_absolute_value=True)
            lmax = cpool.tile([128, 1], F32)
            nc.vector.tensor_reduce(lmax[:], stats[:], mybir.AxisListType.X,
                                    mybir.AluOpType.max)
            pmax = cpool.tile([128, 1], F32)
            nc.gpsimd.partition_all_reduce(pmax[:], lmax[:], 128,
                                           bass_isa.ReduceOp.max)
            cc_in = dpool.tile([1, 1], F32)
            cc_out = dpool.tile([1, 1], F32)
            nc.sync.dma_start(cc_in[:], pmax[0:1, :])
            if single:
                nc.sync.dma_start(cc_out[:], cc_in[:])
            else:
                nc.gpsimd.collective_compute(
                    "AllReduce", mybir.AluOpType.max,
                    replica_groups=[list(range(N_CORES))],
                    ins=[cc_in.opt()], outs=[cc_out.opt()],
                )
            gsb = cpool.tile([1, 1], F32)
            nc.sync.dma_start(gsb[:], cc_out[:])
            gbc = cpool.tile([128, 1], F32)
            nc.gpsimd.partition_broadcast(gbc[:], gsb[:])
            s_a = cpool.tile([128, 1], F32)
            nc.vector.tensor_scalar(s_a[:], gbc[:], 1.0 / 127.0, None,
                                    mybir.AluOpType.mult)
            inv_a = cpool.tile([128, 1], F32)
            nc.vector.reciprocal(inv_a[:], s_a[:])
            s_row = cpool.tile([1, GLOC], F32)
            a2, b2 = bass.broadcast_tensor_aps(sw_sb[:], s_a[0:1, :])
            nc.vector.tensor_tensor(s_row[:], a2, b2, mybir.AluOpType.mult)
            nc.sync.dma_start(oscale[:], s_row[:])
            s_bc = cpool.tile([128, GLOC], F32)
            nc.gpsimd.partition_broadcast(s_bc[:], s_row[:])

            def emit_p2a_one(h, bl):
                b = h * HB + bl
                xt2 = xpool.tile([128, L], F32, tag="x1", name=f"x2_{b}")
                nc.sync.dma_start(xt2[:], x_in[b])
                mkh = wpool.tile([128, L], BF16, tag="mkf", name=f"mkh_{b}")
                nc.vector.tensor_scalar(mkh[:], iof[:], lnf[:, b:b + 1],
                                        None, mybir.AluOpType.is_lt)
                t1 = wpool.tile([128, L], F32, tag="xm", name=f"t1_{b}")
                nc.vector.tensor_scalar(t1[:], xt2[:], inv_a[:], CMAGIC,
                                        mybir.AluOpType.mult,
                                        mybir.AluOpType.add)
                qb = wpool.tile([128, L], BF16, tag="qb", name=f"qb_{b}")
                nc.vector.tensor_scalar(qb[:], t1[:], CMAGIC, None,
                                        mybir.AluOpType.subtract)
                xp = qpool.tile([128, PADL], BF16, tag="xp", name=f"xp_{b}")
                nc.gpsimd.memset(xp[:, 0:PAD], 0)
                nc.gpsimd.memset(xp[:, PAD + L:PADL], 0)
                tt2.tensor_tensor(xp[:, PAD:PAD + L], qb[:], mkh[:],
                                  mybir.AluOpType.mult)
                nc.sync.dma_start(
                    xq_h[h][:, bl * J:(bl + 1) * J, :].rearrange(
                        "g j l -> j g l"),
                    xp[:])

            def emit_p2b_one(h, g):
                xt = tpool.tile([128, NCHK, 128], BF16, tag="xt",
                                name=f"xt_{h}_{g}")
                nc.sync.dma_start_transpose(xt[:], xq_h[h][g])
                A = bands[:, g, 0, :]
                Bm = bands[:, g, 1, :]
                obufs = None
                ps2 = None
                for c in range(OCHK):
                    ci = c % 2
                    pt = ppool.tile([128, 128], F32, tag=f"pt{ci}",
                                    name=f"pt{ci}_{h}_{g}_{c}")
                    nc.tensor.matmul(pt[:], A, xt[:, c, :],
                                     start=True, stop=False)
                    nc.tensor.matmul(pt[:], Bm, xt[:, c + 1, :],
                                     start=False, stop=True)
                    obt = otpool.tile([128, 128], F32, tag="obt",
                                      name=f"obt_{h}_{g}_{c}")
                    nc.vector.tensor_copy(obt[:], pt[:])
                    if c % 2 == 0:
                        ps2 = ptpool.tile([128, 256], F32, tag="ps2",
                                          name=f"ps2_{h}_{g}_{c // 2}")
                    nc.tensor.transpose(ps2[:, ci * 128:(ci + 1) * 128],
                                        obt[:], identf[:])
                    if c % 4 == 0:
                        obufs = opool.tile([128, 512], F32, tag="obuf",
                                           name=f"obuf_{h}_{g}_{c // 4}")
                    if c % 2 == 1:
                        nc.scalar.activation(
                            obufs[:, (c // 2 % 2) * 256:
                                  (c // 2 % 2 + 1) * 256],
                            ps2[:],
                            mybir.ActivationFunctionType.Copy,
                            scale=s_bc[:, g:g + 1])
                    if c % 4 == 3:
                        nc.sync.dma_start(
                            out[h * HB:(h + 1) * HB, :, g,
                                (c // 4) * 512:(c // 4 + 1) * 512],
                            obufs[:])

            if do_p2a:
                for bl in range(HB):
                    emit_p2a_one(0, bl)
            if do_p2a and do_p2b:
                for k in range(HB):
                    emit_p2a_one(1, k)
                    emit_p2b_one(0, k)
                for g in range(GLOC):
                    emit_p2b_one(1, g)
            elif do_p2a:
                for bl in range(HB):
                    emit_p2a_one(1, bl)
            elif do_p2b:
                for h in range(2):
                    for g in range(GLOC):
                        emit_p2b_one(h, g)

    nc.compile()
    return nc


def _host_prep(weight):
    w = weight.reshape(HEADS, K).astype(np.float32)
    s_w = (np.abs(w).max(axis=1, keepdims=True) / np.float32(127.0)).astype(np.float32)
    qw = np.round((w / s_w).astype(np.float32))
    i_idx = np.arange(128)[:, None]
    l_idx = np.arange(128)[None, :]
    d_a = i_idx - l_idx
    d_b = 128 + i_idx - l_idx
    bands = np.zeros((128, HEADS, 2, 128), np.float32)
    va = (d_a >= 0) & (d_a <= 32)
    vb = (d_b >= 0) & (d_b <= 32)
    for g in range(HEADS):
        bands[:, g, 0, :][va] = qw[g][d_a[va]]
        bands[:, g, 1, :][vb] = qw[g][d_b[vb]]
    return s_w.reshape(-1), bands.astype(ml_dtypes.bfloat16)


def kernel(x: np.ndarray, lens: np.ndarray, weight: np.ndarray):
    global _COMPILED, LAST_RESULTS
    if _COMPILED is None:
        _COMPILED = _build()
    nc = _COMPILED

    x = np.ascontiguousarray(x, dtype=np.float32)
    lens = np.ascontiguousarray(lens, dtype=np.int32)
    weight = np.ascontiguousarray(weight, dtype=np.float32)
    s_w, bands = _host_prep(weight)

    xr = x.reshape(B, J, HEADS, L)
    in_maps = []
    for i in range(N_CORES):
        in_maps.append({
            "x": np.ascontiguousarray(xr[:, :, i * GLOC:(i + 1) * GLOC, :]),
            "lens": lens.reshape(1, B),
            "bands": np.ascontiguousarray(bands[:, i * GLOC:(i + 1) * GLOC]),
            "sw": s_w[i * GLOC:(i + 1) * GLOC].reshape(1, GLOC),
        })
    LAST_RESULTS = run_bass_kernel_spmd(nc, in_maps, list(range(N_CORES)))
    res = LAST_RESULTS.results
    outf = np.empty((B, J, HEADS, L), np.float32)
    for i in range(N_CORES):
        outf[:, :, i * GLOC:(i + 1) * GLOC, :] = res[i]["out"]
    out = outf.reshape(B, C, L)
    out_scale = np.concatenate(
        [res[i]["oscale"].reshape(-1) for i in range(N_CORES)])
    out_lens = ((lens + 2 * PAD - (K - 1) - 1) // 1 + 1).astype(np.int32)
    return out, out_lens, out_scale
